# revision 9
# baseline (speedup 1.0000x reference)
"""Trainium2 Bass kernel: causal multi-head attention with RoPE.

Model: B=4, L=2048, H=2048, NH=16 heads, head_dim=128.
  q = x @ Wq.T ; k = x @ Wk.T ; v = x @ Wv.T        (per-head split)
  q, k <- RoPE(q, k)
  attn = softmax(mask(q k^T / sqrt(hd)))
  out  = (attn @ v) heads-concat @ Wo.T

Sharding (8 cores): hybrid batch x tensor-parallel.  Core c handles
batch b = c//2 and heads half*8..half*8+7 with half = c%2.  Wq/Wk/Wv are
column-sharded (8 heads per core), Wo row-sharded; each core produces a
partial y[b] and the host sums the two partials per batch (the unshard
step) and concatenates batches.

Per-core dataflow (all SBUF-resident, bf16 inputs / fp32 accumulation):
  phase A: Q^T, K^T  [128d x 2048pos] per head (d-major) and V
           [128pos x 1024d] pos-major, via PE matmuls; RoPE on Q^T/K^T.
  phase B: flash-style causal attention per (head, 512-wide q chunk):
           S^T tile = K_blk^T Q_chunk (PE), P = exp(S^T/sqrt(d)) (ACT),
           block-sparse causal structure with a triangular-mask multiply
           on diagonal blocks (DVE), O^T += V_blk P (PE), rowsum via
           ones-matmul (PE), reciprocal+broadcast+scale for the softmax
           normalization (DVE + GPSIMD).
  phase C: y^T partial = Wo_shard O^T (PE) -> DRAM fp32.
"""

import math
import numpy as np

B, L, H, NH, HD = 4, 2048, 2048, 16, 128
ROPE_BASE = 10000.0
NCORES = 8
HPC = 8          # heads per core
QC = 512         # q chunk width
NQC = L // QC    # 4 q chunks
NKB = L // 128   # 16 kp blocks
SCALE = 1.0 / math.sqrt(HD)

_cache = {}


def _analyze_mask(mask2d):
    """Classify each (q_block, kp_block) 128x128 block of the [L, L] mask.

    Returns (block_kind[16][16] with 0=empty,1=full,2=mixed, patterns,
    pattern_idx dict keyed by block coords). mask2d is int32 [L, L],
    rows=q, cols=kp.
    """
    nb = L // 128
    kind = [[0] * nb for _ in range(nb)]
    patterns = []
    pat_key_to_idx = {}
    block_pat = {}
    for qb in range(nb):
        rows = mask2d[qb * 128:(qb + 1) * 128]
        for kb in range(nb):
            blk = rows[:, kb * 128:(kb + 1) * 128]
            s = int(blk.sum())
            if s == 0:
                kind[qb][kb] = 0
            elif s == 128 * 128:
                kind[qb][kb] = 1
            else:
                kind[qb][kb] = 2
                key = blk.tobytes()
                idx = pat_key_to_idx.get(key)
                if idx is None:
                    idx = len(patterns)
                    pat_key_to_idx[key] = idx
                    # stored transposed: S^T tiles are [kp, q]
                    patterns.append(np.ascontiguousarray(blk.T))
                block_pat[(qb, kb)] = idx
    return kind, patterns, block_pat


def _build(kind, block_pat, n_patterns):
    """Build the SPMD bass program (same for all 8 cores)."""
    import concourse.bass as bass
    import concourse.bacc as bacc
    import concourse.mybir as mybir
    import concourse.tile as tile

    fp32 = mybir.dt.float32
    bf16 = mybir.dt.bfloat16
    EXP = mybir.ActivationFunctionType.Exp

    nc = bacc.Bacc("TRN2", target_bir_lowering=False, debug=False)

    xT = nc.dram_tensor("xT", [H, L], bf16, kind="ExternalInput")
    wqT = nc.dram_tensor("wqT", [H, HPC * HD], bf16, kind="ExternalInput")
    wkT = nc.dram_tensor("wkT", [H, HPC * HD], bf16, kind="ExternalInput")
    wvT = nc.dram_tensor("wvT", [H, HPC * HD], bf16, kind="ExternalInput")
    woT = nc.dram_tensor("woT", [HPC * HD, H], bf16, kind="ExternalInput")
    cosd = nc.dram_tensor("cosd", [HD, L], bf16, kind="ExternalInput")
    sinmd = nc.dram_tensor("sinmd", [HD, L], bf16, kind="ExternalInput")
    rotd = nc.dram_tensor("rotd", [HD, HD], bf16, kind="ExternalInput")
    npat = max(n_patterns, 1)
    maskd = nc.dram_tensor("maskd", [npat, 128, 128], bf16, kind="ExternalInput")
    yT = nc.dram_tensor("yT", [H, L], fp32, kind="ExternalOutput")

    NHC = H // 128  # 16 input-feature blocks

    def proj_phase(tc, w_dram, out_tiles, xpool, wtag, dmajor,
                   rope_refs=None):
        """dmajor=True: out[d, pos] per head tiles; False: V pos-major.

        rope_refs = (rot_sb, cos_sb, sinm_sb) to apply RoPE per chunk:
        the rotate-half partition shuffle runs on PE via a signed
        permutation matmul (DVE cannot cross partitions), then
        q' = q*cos + rot(q)*sinm on DVE, all partition-aligned.
        """
        with tc.tile_pool(name=f"w_{wtag}", bufs=1) as wpool, \
             tc.tile_pool(name=f"t_{wtag}", bufs=3) as tpool, \
             tc.tile_pool(name=f"ps_{wtag}", bufs=4, space="PSUM") as pspool:
            w_sb = wpool.tile([128, NHC, HPC * HD], bf16, tag=f"w_{wtag}")
            nc.sync.dma_start(
                out=w_sb[:],
                in_=w_dram[:].rearrange("(a p) m -> p a m", p=128))
            for j in range(NQC):
                x_sb = xpool.tile([128, NHC, QC], bf16, tag="xcols")
                nc.sync.dma_start(
                    out=x_sb[:],
                    in_=xT[:, j * QC:(j + 1) * QC].rearrange(
                        "(a p) m -> p a m", p=128))
                if dmajor:
                    rot_sb, cos_sb, sinm_sb = rope_refs
                    js = slice(j * QC, (j + 1) * QC)
                    for h in range(HPC):
                        ps = pspool.tile([128, QC], fp32, tag=f"ps_{wtag}")
                        for hc in range(NHC):
                            nc.tensor.matmul(
                                ps[:],
                                w_sb[:, hc, h * HD:(h + 1) * HD],
                                x_sb[:, hc, :],
                                start=(hc == 0), stop=(hc == NHC - 1))
                        q = out_tiles[h][:, js]
                        nc.scalar.copy(q, ps[:])
                        psrot = pspool.tile([128, QC], fp32,
                                            tag=f"rot_{wtag}", bufs=2)
                        nc.tensor.matmul(psrot[:], rot_sb[:], q,
                                         start=True, stop=True)
                        tmp = tpool.tile([128, QC], bf16, tag="ropetmp")
                        nc.vector.tensor_mul(tmp[:], psrot[:],
                                             sinm_sb[:, js])
                        nc.vector.tensor_mul(q, q, cos_sb[:, js])
                        nc.vector.tensor_add(q, q, tmp[:])
                else:
                    for pb in range(4):      # pos blocks within chunk
                        for dc in range(2):  # 512-wide d chunks
                            ps = pspool.tile([128, QC], fp32, tag=f"ps_{wtag}")
                            for hc in range(NHC):
                                nc.tensor.matmul(
                                    ps[:],
                                    x_sb[:, hc, pb * 128:(pb + 1) * 128],
                                    w_sb[:, hc, dc * QC:(dc + 1) * QC],
                                    start=(hc == 0), stop=(hc == NHC - 1))
                            nc.scalar.copy(
                                out_tiles[j * 4 + pb][:, dc * QC:(dc + 1) * QC],
                                ps[:])

    with tile.TileContext(nc) as tc:
        with tc.tile_pool(name="persist", bufs=1) as persist:
            cos_sb = persist.tile([HD, L], bf16, tag="cos")
            sinm_sb = persist.tile([HD, L], bf16, tag="sinm")
            nc.sync.dma_start(out=cos_sb[:], in_=cosd[:])
            nc.sync.dma_start(out=sinm_sb[:], in_=sinmd[:])
            ones_sb = persist.tile([128, 1], bf16, tag="ones")
            nc.vector.memset(ones_sb[:], 1.0)
            rot_sb = persist.tile([HD, HD], bf16, tag="rot")
            nc.sync.dma_start(out=rot_sb[:], in_=rotd[:])
            mt_sb = []
            for p in range(n_patterns):
                mt = persist.tile([128, 128], bf16, tag=f"mt{p}")
                nc.sync.dma_start(out=mt[:], in_=maskd[p])
                mt_sb.append(mt)

            QTa = persist.tile([HD, HPC, L], bf16, tag="qta")
            KTa = persist.tile([HD, HPC, L], bf16, tag="kta")
            Va = persist.tile([128, NKB, HPC * HD], bf16, tag="va")
            QT = [QTa[:, h, :] for h in range(HPC)]
            KT = [KTa[:, h, :] for h in range(HPC)]
            V = [Va[:, i, :] for i in range(NKB)]

            # ---------------- phase A: projections + RoPE ----------------
            rope_refs = (rot_sb, cos_sb, sinm_sb)
            with tc.tile_pool(name="xpool", bufs=2) as xpool:
                proj_phase(tc, wqT, QT, xpool, "q", True, rope_refs)
                proj_phase(tc, wkT, KT, xpool, "k", True, rope_refs)
                proj_phase(tc, wvT, V, xpool, "v", False)

            with tc.tile_pool(name="otp", bufs=1) as otp:
                OTa = otp.tile([HD, HPC, L], bf16, tag="ota")
                OT = [OTa[:, h, :] for h in range(HPC)]

                # ---------------- phase B: attention ----------------
                with tc.tile_pool(name="pp", bufs=4) as ppool, \
                     tc.tile_pool(name="rr", bufs=2) as rpool, \
                     tc.tile_pool(name="bb", bufs=2) as bpool, \
                     tc.tile_pool(name="ps_s", bufs=3, space="PSUM") as ps_s, \
                     tc.tile_pool(name="ps_o", bufs=2, space="PSUM") as ps_o, \
                     tc.tile_pool(name="ps_r", bufs=2, space="PSUM") as ps_r:
                    for h in range(HPC):
                        for j in range(NQC):
                            # kp blocks needed for this q chunk
                            blocks = []
                            for i in range(NKB):
                                live = [t for t in range(4)
                                        if kind[4 * j + t][i] != 0]
                                if live:
                                    blocks.append((i, live))
                            if not blocks:
                                continue
                            pso = ps_o.tile([128, QC], fp32, tag="pso")
                            psr = ps_r.tile([1, QC], fp32, tag="psr")
                            last = len(blocks) - 1
                            for bi, (i, live) in enumerate(blocks):
                                t0, t1 = live[0], live[-1]
                                w0, w1 = t0 * 128, (t1 + 1) * 128
                                pss = ps_s.tile([128, QC], fp32, tag="pss")
                                nc.tensor.matmul(
                                    pss[:, w0:w1],
                                    KT[h][:, i * 128:(i + 1) * 128],
                                    QT[h][:, j * QC + w0:j * QC + w1],
                                    start=True, stop=True)
                                P = ppool.tile([128, QC], bf16, tag="p")
                                if w0 > 0:
                                    nc.vector.memset(P[:, 0:w0], 0.0)
                                if w1 < QC:
                                    nc.vector.memset(P[:, w1:QC], 0.0)
                                nc.scalar.activation(P[:, w0:w1],
                                                     pss[:, w0:w1],
                                                     EXP, scale=SCALE)
                                for t in range(t0, t1 + 1):
                                    qb = 4 * j + t
                                    if kind[qb][i] == 0:
                                        nc.vector.memset(
                                            P[:, t * 128:(t + 1) * 128], 0.0)
                                    elif kind[qb][i] == 2:
                                        mt = mt_sb[block_pat[(qb, i)]]
                                        nc.vector.tensor_mul(
                                            P[:, t * 128:(t + 1) * 128],
                                            P[:, t * 128:(t + 1) * 128],
                                            mt[:])
                                nc.tensor.matmul(
                                    pso[:], V[i][:, h * HD:(h + 1) * HD],
                                    P[:],
                                    start=(bi == 0), stop=(bi == last))
                                nc.tensor.matmul(
                                    psr[:], ones_sb[:], P[:],
                                    start=(bi == 0), stop=(bi == last))
                            r_sb = rpool.tile([1, QC], fp32, tag="r")
                            nc.vector.reciprocal_approx_fast(out=r_sb[:],
                                                             in_=psr[:])
                            bc_sb = bpool.tile([128, QC], fp32, tag="bc")
                            nc.gpsimd.partition_broadcast(bc_sb[:], r_sb[:])
                            nc.vector.tensor_mul(
                                OT[h][:, j * QC:(j + 1) * QC], pso[:],
                                bc_sb[:])

                # ---------------- phase C: output projection ----------------
                with tc.tile_pool(name="wo", bufs=1) as wop, \
                     tc.tile_pool(name="ysb", bufs=3) as ypool, \
                     tc.tile_pool(name="ps_c", bufs=4, space="PSUM") as ps_c:
                    wo_sb = wop.tile([128, HPC, H], bf16, tag="wo")
                    nc.sync.dma_start(
                        out=wo_sb[:],
                        in_=woT[:].rearrange("(a p) m -> p a m", p=128))
                    for oc in range(H // 128):
                        for j in range(NQC):
                            ps = ps_c.tile([128, QC], fp32, tag="psc")
                            for fc in range(HPC):
                                nc.tensor.matmul(
                                    ps[:],
                                    wo_sb[:, fc, oc * 128:(oc + 1) * 128],
                                    OT[fc][:, j * QC:(j + 1) * QC],
                                    start=(fc == 0), stop=(fc == HPC - 1))
                            y_sb = ypool.tile([128, QC], fp32, tag="y")
                            nc.vector.tensor_copy(y_sb[:], ps[:])
                            nc.sync.dma_start(
                                out=yT[oc * 128:(oc + 1) * 128,
                                       j * QC:(j + 1) * QC],
                                in_=y_sb[:])

    nc.compile()
    return nc


def _prep_inputs(x, mask, Wq, Wk, Wv, Wo, patterns):
    import ml_dtypes
    bf16 = ml_dtypes.bfloat16

    # RoPE tables, d-major [HD, L]
    inv_freq = 1.0 / (ROPE_BASE ** (np.arange(0, HD, 2, dtype=np.float64)
                                    / HD))
    t = np.arange(L, dtype=np.float64)
    freqs = np.outer(t, inv_freq)                     # [L, HD/2]
    emb = np.concatenate((freqs, freqs), axis=-1)     # [L, HD]
    cos = np.cos(emb).T.astype(np.float32)            # [HD, L]
    sin = np.sin(emb).T.astype(np.float32)
    sinm = sin.copy()
    sinm[0:64] = -sin[0:64]
    cos_b = cos.astype(bf16)
    sinm_b = sinm.astype(bf16)

    # rotate-half permutation (signs folded into sinm already except the
    # order: rot(q)[d] = -q[d+64] for d<64, +q[d-64] for d>=64; the minus
    # lives in sinm, so the matrix here is unsigned position swap)
    rot = np.zeros((HD, HD), dtype=np.float32)
    for d in range(64):
        rot[d + 64, d] = 1.0
        rot[d, d + 64] = 1.0
    rot_b = rot.astype(bf16)

    npat = max(len(patterns), 1)
    maskd = np.zeros((npat, 128, 128), dtype=bf16)
    for i, p in enumerate(patterns):
        maskd[i] = p.astype(np.float32).astype(bf16)

    in_maps = []
    for c in range(NCORES):
        b, half = c // 2, c % 2
        rows = slice(half * HPC * HD, (half + 1) * HPC * HD)
        in_maps.append({
            "xT": np.ascontiguousarray(x[b].T).astype(bf16),
            "wqT": np.ascontiguousarray(Wq[rows, :].T).astype(bf16),
            "wkT": np.ascontiguousarray(Wk[rows, :].T).astype(bf16),
            "wvT": np.ascontiguousarray(Wv[rows, :].T).astype(bf16),
            "woT": np.ascontiguousarray(Wo[:, rows].T).astype(bf16),
            "cosd": cos_b,
            "sinmd": sinm_b,
            "rotd": rot_b,
            "maskd": maskd,
        })
    return in_maps


def kernel(x, mask, Wq, Wk, Wv, Wo, _trace=False):
    from concourse.bass_utils import run_bass_kernel_spmd

    x = np.asarray(x, dtype=np.float32)
    mask2d = np.asarray(mask, dtype=np.int32).reshape(L, L)
    key = mask2d.tobytes()
    if key not in _cache:
        kind, patterns, block_pat = _analyze_mask(mask2d)
        nc = _build(kind, block_pat, len(patterns))
        _cache[key] = (nc, patterns)
    nc, patterns = _cache[key]

    in_maps = _prep_inputs(x, mask, np.asarray(Wq, np.float32),
                           np.asarray(Wk, np.float32),
                           np.asarray(Wv, np.float32),
                           np.asarray(Wo, np.float32), patterns)
    res = run_bass_kernel_spmd(nc, in_maps, list(range(NCORES)),
                               trace=_trace)
    y = np.empty((B, L, H), dtype=np.float32)
    for b in range(B):
        acc = res.results[2 * b]["yT"].astype(np.float32) + \
              res.results[2 * b + 1]["yT"].astype(np.float32)
        y[b] = acc.T
    if _trace:
        kernel.last_results = res
    return y


if __name__ == "__main__":
    import reference
    inputs = reference.setup_inputs()
    inputs = {k: np.asarray(v) for k, v in inputs.items()}
    out = kernel(**inputs)
    exp = np.asarray(reference.reference(**{k: v for k, v in inputs.items()}))
    err = np.abs(out - exp).max() / np.abs(exp).max()
    print("rel err (absmax):", err)


# revision 14
# speedup vs baseline: 1.1134x; 1.1134x over previous
"""Trainium2 Bass kernel: causal multi-head attention with RoPE.

Model: B=4, L=2048, H=2048, NH=16 heads, head_dim=128.
  q = x @ Wq.T ; k = x @ Wk.T ; v = x @ Wv.T        (per-head split)
  q, k <- RoPE(q, k)
  attn = softmax(mask(q k^T / sqrt(hd)))
  out  = (attn @ v) heads-concat @ Wo.T

Sharding (8 cores): hybrid batch x tensor-parallel.  Core c handles
batch b = c//2 and heads half*8..half*8+7 with half = c%2.  Wq/Wk/Wv are
column-sharded (8 heads per core), Wo row-sharded; each core produces a
partial y[b] and the host sums the two partials per batch (the unshard
step) and concatenates batches.

Per-core dataflow (all SBUF-resident, bf16 inputs / fp32 accumulation):
  phase A: Q^T, K^T  [128d x 2048pos] per head (d-major) and V
           [128pos x 1024d] pos-major, via PE matmuls; RoPE on Q^T/K^T
           (rotate-half partition shuffle via SBUF->SBUF DMA, the
           elementwise part on DVE).
  phase B: flash-style causal attention per (head, 512-wide q chunk):
           S^T tile = K_blk^T Q_chunk (PE), P = exp(S^T/sqrt(d)) (ACT),
           block-sparse causal structure with a triangular-mask multiply
           on diagonal blocks (DVE), O^T += V_blk P (PE), rowsum via
           ones-matmul (PE), reciprocal+broadcast+scale for the softmax
           normalization (DVE + GPSIMD).
  phase C: y^T partial = Wo_shard O^T (PE) -> DRAM fp32.
"""

import math
import numpy as np

B, L, H, NH, HD = 4, 2048, 2048, 16, 128
ROPE_BASE = 10000.0
NCORES = 8
HPC = 8          # heads per core
QC = 512         # q chunk width
NQC = L // QC    # 4 q chunks
NKB = L // 128   # 16 kp blocks
SCALE = 1.0 / math.sqrt(HD)

_cache = {}


def _analyze_mask(mask2d):
    """Classify each (q_block, kp_block) 128x128 block of the [L, L] mask.

    Returns (block_kind[16][16] with 0=empty,1=full,2=mixed, patterns,
    pattern_idx dict keyed by block coords). mask2d is int32 [L, L],
    rows=q, cols=kp.
    """
    nb = L // 128
    kind = [[0] * nb for _ in range(nb)]
    patterns = []
    pat_key_to_idx = {}
    block_pat = {}
    for qb in range(nb):
        rows = mask2d[qb * 128:(qb + 1) * 128]
        for kb in range(nb):
            blk = rows[:, kb * 128:(kb + 1) * 128]
            s = int(blk.sum())
            if s == 0:
                kind[qb][kb] = 0
            elif s == 128 * 128:
                kind[qb][kb] = 1
            else:
                kind[qb][kb] = 2
                key = blk.tobytes()
                idx = pat_key_to_idx.get(key)
                if idx is None:
                    idx = len(patterns)
                    pat_key_to_idx[key] = idx
                    # stored transposed: S^T tiles are [kp, q]
                    patterns.append(np.ascontiguousarray(blk.T))
                block_pat[(qb, kb)] = idx
    return kind, patterns, block_pat


def _build(kind, block_pat, n_patterns):
    """Build the SPMD bass program (same for all 8 cores)."""
    import concourse.bass as bass
    import concourse.bacc as bacc
    import concourse.mybir as mybir
    import concourse.tile as tile

    fp32 = mybir.dt.float32
    bf16 = mybir.dt.bfloat16
    EXP = mybir.ActivationFunctionType.Exp

    nc = bacc.Bacc("TRN2", target_bir_lowering=False, debug=False)

    xT = nc.dram_tensor("xT", [H, L], bf16, kind="ExternalInput")
    wqT = nc.dram_tensor("wqT", [H, HPC * HD], bf16, kind="ExternalInput")
    wkT = nc.dram_tensor("wkT", [H, HPC * HD], bf16, kind="ExternalInput")
    wvT = nc.dram_tensor("wvT", [H, HPC * HD], bf16, kind="ExternalInput")
    woT = nc.dram_tensor("woT", [HPC * HD, H], bf16, kind="ExternalInput")
    cosd = nc.dram_tensor("cosd", [HD, L], bf16, kind="ExternalInput")
    sinmd = nc.dram_tensor("sinmd", [HD, L], bf16, kind="ExternalInput")
    npat = max(n_patterns, 1)
    maskd = nc.dram_tensor("maskd", [npat, 128, 128], bf16, kind="ExternalInput")
    yT = nc.dram_tensor("yT", [H, L], fp32, kind="ExternalOutput")

    NHC = H // 128  # 16 input-feature blocks

    def qk_phase(tc, w_dram, out_a, wpool, xpool, tpool, pspool, wtag,
                 cos_sb, sinm_sb):
        """Q^T / K^T d-major projection + fused RoPE per (head, chunk)."""
        w_sb = wpool.tile([128, NHC, HPC * HD], bf16, tag="w",
                          name=f"w_{wtag}")
        nc.sync.dma_start(
            out=w_sb[:], in_=w_dram[:].rearrange("(a p) m -> p a m", p=128))
        for j in range(NQC):
            js = slice(j * QC, (j + 1) * QC)
            x_sb = xpool.tile([128, NHC, QC], bf16, tag="xcols",
                              name=f"x_{wtag}{j}")
            nc.sync.dma_start(
                out=x_sb[:],
                in_=xT[:, js].rearrange("(a p) m -> p a m", p=128))
            for h in range(HPC):
                ps = pspool.tile([128, QC], fp32, tag="ps_proj")
                for hc in range(NHC):
                    nc.tensor.matmul(
                        ps[:],
                        w_sb[:, hc, h * HD:(h + 1) * HD],
                        x_sb[:, hc, :],
                        start=(hc == 0), stop=(hc == NHC - 1))
                q = out_a[:, h, js]
                nc.scalar.copy(q, ps[:])
                # rotate-half: pure partition swap, done by SBUF->SBUF DMA
                rq = tpool.tile([128, QC], bf16, tag="rotq")
                nc.sync.dma_start(out=rq[0:64, :], in_=out_a[64:128, h, js])
                nc.sync.dma_start(out=rq[64:128, :], in_=out_a[0:64, h, js])
                nc.vector.tensor_mul(rq[:], rq[:], sinm_sb[:, js])
                nc.vector.tensor_mul(q, q, cos_sb[:, js])
                nc.vector.tensor_add(q, q, rq[:])

    def v_phase(tc, w_dram, va, wpool, xpool, pspool):
        """V pos-major projection (x chunks 256 wide to fit SBUF)."""
        w_sb = wpool.tile([128, NHC, HPC * HD], bf16, tag="w", name="w_v")
        nc.sync.dma_start(
            out=w_sb[:], in_=w_dram[:].rearrange("(a p) m -> p a m", p=128))
        VC = 256
        for j in range(L // VC):
            x_sb = xpool.tile([128, NHC, VC], bf16, tag="xv", name=f"xv{j}")
            nc.sync.dma_start(
                out=x_sb[:],
                in_=xT[:, j * VC:(j + 1) * VC].rearrange(
                    "(a p) m -> p a m", p=128))
            for pb in range(VC // 128):
                for dc in range(2):
                    ps = pspool.tile([128, QC], fp32, tag="ps_proj")
                    for hc in range(NHC):
                        nc.tensor.matmul(
                            ps[:],
                            x_sb[:, hc, pb * 128:(pb + 1) * 128],
                            w_sb[:, hc, dc * QC:(dc + 1) * QC],
                            start=(hc == 0), stop=(hc == NHC - 1))
                    nc.scalar.copy(
                        va[:, j * (VC // 128) + pb, dc * QC:(dc + 1) * QC],
                        ps[:])

    with tile.TileContext(nc) as tc:
        with tc.tile_pool(name="persist", bufs=1, side="left") as persist:
            # one combined small-constant tile: [trimask patterns | ones]
            cst = persist.tile([128, npat * 128 + 8], bf16, tag="cst")
            for p in range(n_patterns):
                nc.sync.dma_start(out=cst[:, p * 128:(p + 1) * 128],
                                  in_=maskd[p])
            ones_col = npat * 128
            nc.vector.memset(cst[:, ones_col:ones_col + 1], 1.0)
            QTa = persist.tile([HD, HPC, L], bf16, tag="qta")
            KTa = persist.tile([HD, HPC, L], bf16, tag="kta")

            # ---------------- phase A: projections + RoPE ----------------
            # Manual pool lifetimes (non-LIFO): weights/x/rope tables are
            # freed before attention while Va spans V-phase..attention.
            wpool_cm = tc.tile_pool(name="wpool", bufs=2, side="right")
            wpool = wpool_cm.__enter__()
            ropec_cm = tc.tile_pool(name="ropec", bufs=1, side="right")
            ropec = ropec_cm.__enter__()
            psp_cm = tc.tile_pool(name="ps_proj", bufs=4, space="PSUM")
            psp = psp_cm.__enter__()

            cos_sb = ropec.tile([HD, L], bf16, tag="cos")
            sinm_sb = ropec.tile([HD, L], bf16, tag="sinm")
            nc.sync.dma_start(out=cos_sb[:], in_=cosd[:])
            nc.sync.dma_start(out=sinm_sb[:], in_=sinmd[:])

            xqk_cm = tc.tile_pool(name="xqk", bufs=2, side="right")
            xqk = xqk_cm.__enter__()
            tpool_cm = tc.tile_pool(name="tpool", bufs=3, side="right")
            tpool = tpool_cm.__enter__()
            qk_phase(tc, wqT, QTa, wpool, xqk, tpool, psp, "q",
                     cos_sb, sinm_sb)
            qk_phase(tc, wkT, KTa, wpool, xqk, tpool, psp, "k",
                     cos_sb, sinm_sb)
            tpool_cm.__exit__(None, None, None)
            xqk_cm.__exit__(None, None, None)
            ropec_cm.__exit__(None, None, None)

            vp_cm = tc.tile_pool(name="vp", bufs=1, side="left")
            vp_outer = vp_cm.__enter__()
            Va = vp_outer.tile([128, NKB, HPC * HD], bf16, tag="va")
            xv_cm = tc.tile_pool(name="xv", bufs=2, side="right")
            xv = xv_cm.__enter__()
            v_phase(tc, wvT, Va, wpool, xv, psp)
            xv_cm.__exit__(None, None, None)
            wpool_cm.__exit__(None, None, None)
            psp_cm.__exit__(None, None, None)

            # -------- phase B + C under Va's lifetime --------
            _attn_and_out(tc, nc, kind, block_pat, QTa, KTa, Va,
                          cst, ones_col, woT, yT, fp32, bf16, EXP)
            vp_cm.__exit__(None, None, None)

    nc.compile()
    return nc


def _attn_and_out(tc, nc, kind, block_pat, QTa, KTa, Va, cst, ones_col,
                  woT, yT, fp32, bf16, EXP):
    ones_sb = cst[:, ones_col:ones_col + 1]
    with tc.tile_pool(name="otp", bufs=1, side="left") as otp, \
         tc.tile_pool(name="wo", bufs=1, side="left") as wop:
        OTa = otp.tile([HD, HPC, L], bf16, tag="ota")
        wo_sb = wop.tile([128, HPC, H], bf16, tag="wo")
        # prefetch Wo during attention
        nc.sync.dma_start(
            out=wo_sb[:], in_=woT[:].rearrange("(a p) m -> p a m", p=128))

        # ---------------- phase B: attention ----------------
        with tc.tile_pool(name="pp", bufs=4, side="right") as ppool, \
             tc.tile_pool(name="rr", bufs=2, side="right") as rpool, \
             tc.tile_pool(name="bb", bufs=2, side="right") as bpool, \
             tc.tile_pool(name="ps_s", bufs=3, space="PSUM") as ps_s, \
             tc.tile_pool(name="ps_o", bufs=2, space="PSUM") as ps_o, \
             tc.tile_pool(name="ps_r", bufs=2, space="PSUM") as ps_r:
            for h in range(HPC):
                for j in range(NQC):
                    blocks = []
                    for i in range(NKB):
                        live = [t for t in range(4)
                                if kind[4 * j + t][i] != 0]
                        if live:
                            blocks.append((i, live))
                    if not blocks:
                        continue
                    pso = ps_o.tile([128, QC], fp32, tag="pso")
                    psr = ps_r.tile([1, QC], fp32, tag="psr")
                    last = len(blocks) - 1
                    for bi, (i, live) in enumerate(blocks):
                        t0, t1 = live[0], live[-1]
                        w0, w1 = t0 * 128, (t1 + 1) * 128
                        pss = ps_s.tile([128, QC], fp32, tag="pss")
                        nc.tensor.matmul(
                            pss[:, w0:w1],
                            KTa[:, h, i * 128:(i + 1) * 128],
                            QTa[:, h, j * QC + w0:j * QC + w1],
                            start=True, stop=True)
                        P = ppool.tile([128, QC], bf16, tag="p")
                        if w0 > 0 and bi == 0:
                            nc.vector.memset(P[:, 0:w0], 0.0)
                        if w1 < QC and bi == 0:
                            nc.vector.memset(P[:, w1:QC], 0.0)
                        nc.scalar.activation(P[:, w0:w1], pss[:, w0:w1],
                                             EXP, scale=SCALE)
                        for t in range(t0, t1 + 1):
                            qb = 4 * j + t
                            if kind[qb][i] == 0:
                                nc.vector.memset(
                                    P[:, t * 128:(t + 1) * 128], 0.0)
                            elif kind[qb][i] == 2:
                                pat = block_pat[(qb, i)]
                                nc.vector.tensor_mul(
                                    P[:, t * 128:(t + 1) * 128],
                                    P[:, t * 128:(t + 1) * 128],
                                    cst[:, pat * 128:(pat + 1) * 128])
                        # first block covers full width (start=True must
                        # touch every psum column); later blocks narrowed
                        m0 = 0 if bi == 0 else w0
                        nc.tensor.matmul(
                            pso[:, m0:QC],
                            Va[:, i, h * HD:(h + 1) * HD],
                            P[:, m0:QC],
                            start=(bi == 0), stop=(bi == last))
                        nc.tensor.matmul(
                            psr[:, m0:QC], ones_sb, P[:, m0:QC],
                            start=(bi == 0), stop=(bi == last))
                    r_sb = rpool.tile([1, QC], fp32, tag="r")
                    nc.vector.reciprocal_approx_fast(out=r_sb[:],
                                                     in_=psr[:])
                    bc_sb = bpool.tile([128, QC], fp32, tag="bc")
                    nc.sync.dma_start(out=bc_sb[:],
                                      in_=r_sb[:].to_broadcast((128, QC)))
                    nc.vector.tensor_mul(
                        OTa[:, h, j * QC:(j + 1) * QC], pso[:], bc_sb[:])

        # ---------------- phase C: output projection ----------------
        with tc.tile_pool(name="ysb", bufs=3, side="right") as ypool, \
             tc.tile_pool(name="ps_c", bufs=4, space="PSUM") as ps_c:
            for oc in range(H // 128):
                for j in range(NQC):
                    ps = ps_c.tile([128, QC], fp32, tag="psc")
                    for fc in range(HPC):
                        nc.tensor.matmul(
                            ps[:],
                            wo_sb[:, fc, oc * 128:(oc + 1) * 128],
                            OTa[:, fc, j * QC:(j + 1) * QC],
                            start=(fc == 0), stop=(fc == HPC - 1))
                    y_sb = ypool.tile([128, QC], fp32, tag="y")
                    nc.vector.tensor_copy(y_sb[:], ps[:])
                    nc.sync.dma_start(
                        out=yT[oc * 128:(oc + 1) * 128,
                               j * QC:(j + 1) * QC],
                        in_=y_sb[:])


def _prep_inputs(x, mask, Wq, Wk, Wv, Wo, patterns):
    import ml_dtypes
    bf16 = ml_dtypes.bfloat16

    # RoPE tables, d-major [HD, L]
    inv_freq = 1.0 / (ROPE_BASE ** (np.arange(0, HD, 2, dtype=np.float64)
                                    / HD))
    t = np.arange(L, dtype=np.float64)
    freqs = np.outer(t, inv_freq)                     # [L, HD/2]
    emb = np.concatenate((freqs, freqs), axis=-1)     # [L, HD]
    cos = np.cos(emb).T.astype(np.float32)            # [HD, L]
    sin = np.sin(emb).T.astype(np.float32)
    sinm = sin.copy()
    sinm[0:64] = -sin[0:64]
    cos_b = cos.astype(bf16)
    sinm_b = sinm.astype(bf16)

    npat = max(len(patterns), 1)
    maskd = np.zeros((npat, 128, 128), dtype=bf16)
    for i, p in enumerate(patterns):
        maskd[i] = p.astype(np.float32).astype(bf16)

    in_maps = []
    for c in range(NCORES):
        b, half = c // 2, c % 2
        rows = slice(half * HPC * HD, (half + 1) * HPC * HD)
        in_maps.append({
            "xT": np.ascontiguousarray(x[b].T).astype(bf16),
            "wqT": np.ascontiguousarray(Wq[rows, :].T).astype(bf16),
            "wkT": np.ascontiguousarray(Wk[rows, :].T).astype(bf16),
            "wvT": np.ascontiguousarray(Wv[rows, :].T).astype(bf16),
            "woT": np.ascontiguousarray(Wo[:, rows].T).astype(bf16),
            "cosd": cos_b,
            "sinmd": sinm_b,
            "maskd": maskd,
        })
    return in_maps


def kernel(x, mask, Wq, Wk, Wv, Wo, _trace=False):
    from concourse.bass_utils import run_bass_kernel_spmd

    x = np.asarray(x, dtype=np.float32)
    mask2d = np.asarray(mask, dtype=np.int32).reshape(L, L)
    key = mask2d.tobytes()
    if key not in _cache:
        kind, patterns, block_pat = _analyze_mask(mask2d)
        nc = _build(kind, block_pat, len(patterns))
        _cache[key] = (nc, patterns)
    nc, patterns = _cache[key]

    in_maps = _prep_inputs(x, mask, np.asarray(Wq, np.float32),
                           np.asarray(Wk, np.float32),
                           np.asarray(Wv, np.float32),
                           np.asarray(Wo, np.float32), patterns)
    res = run_bass_kernel_spmd(nc, in_maps, list(range(NCORES)),
                               trace=_trace)
    y = np.empty((B, L, H), dtype=np.float32)
    for b in range(B):
        acc = res.results[2 * b]["yT"].astype(np.float32) + \
              res.results[2 * b + 1]["yT"].astype(np.float32)
        y[b] = acc.T
    if _trace:
        kernel.last_results = res
    return y


if __name__ == "__main__":
    import reference
    inputs = reference.setup_inputs()
    inputs = {k: np.asarray(v) for k, v in inputs.items()}
    out = kernel(**inputs)
    exp = np.asarray(reference.reference(**{k: v for k, v in inputs.items()}))
    err = np.abs(out - exp).max() / np.abs(exp).max()
    print("rel err (absmax):", err)
